# revision 4
# baseline (speedup 1.0000x reference)
"""MoE feed-forward (top-1 routing) Trainium2 kernel.

Strategy
--------
Pass 1 (device, 8 cores, token-parallel): gate logits^T = Wg^T @ X^T + bg
  in fp16 (PE full rate). Tokens whose top-2 logit margin is below a
  safety threshold (~5e-3, vs a measured worst-case fp16 logit error of
  ~9e-4) are re-decided on the host in float64, so the routing exactly
  matches the fp32 reference argmax.
Host: group token ids by expert; experts are paired (largest with
  smallest count) and each pair is served by 4 cores; each core gets two
  column segments, one per expert. Pure data movement (gather columns of
  X^T, cast fp16).
Pass 2 (device, 8 cores, expert-parallel): per core, per segment:
  Y^T = W2^T @ gelu(W1^T @ X^T + b1) + b2, fp16 operands with fp32 PSUM
  accumulation (PE runs fp16 at 4x the fp32 rate). Weights stream from
  HBM; activations stay resident in SBUF.
Host: scatter rows back into the [B, L, D] output.
"""

import sys

if "/opt/trn_rl_repo" not in sys.path:
    sys.path.insert(0, "/opt/trn_rl_repo")

import numpy as np

import concourse.bacc as bacc
import concourse.mybir as mybir
import concourse.tile as tile

D, F, E = 1024, 4096, 4
B, L = 4, 2048
T = B * L
NC = 8
TPC = T // NC  # tokens per core in the gate pass
P = 128
KD = D // P    # 8  k-tiles over D
KF = F // P    # 32 k-tiles over F

GATE_MARGIN = 5e-3

TRACE = False
LAST_EXEC_NS = []
LAST_TRACES = []

_cache = {}


def _run(nc, in_maps):
    from concourse import bass_utils

    if TRACE:
        bass_utils.upload_artifacts = lambda d: "local://" + d
    res = bass_utils.run_bass_kernel_spmd(
        nc, in_maps, core_ids=list(range(NC)), trace=TRACE
    )
    if TRACE:
        LAST_EXEC_NS.append(res.exec_time_ns)
        LAST_TRACES.append(
            res.instructions_and_trace[1] if res.instructions_and_trace else None
        )
    return res


def _subchunks(C):
    subs = []
    s = 0
    while s < C:
        sz = min(512, C - s)
        subs.append((s, sz))
        s += sz
    return subs


def _build_gate():
    if "gate" in _cache:
        return _cache["gate"]
    f32 = mybir.dt.float32
    f16 = mybir.dt.float16
    nc = bacc.Bacc("TRN2", target_bir_lowering=False, debug=False, num_devices=NC)
    xt = nc.dram_tensor("xt", (D, TPC), f16, kind="ExternalInput")
    wg = nc.dram_tensor("wg", (D, E), f16, kind="ExternalInput")
    bg = nc.dram_tensor("bg", (E, 1), f32, kind="ExternalInput")
    lo = nc.dram_tensor("lo", (E, TPC), f32, kind="ExternalOutput")

    subs = _subchunks(TPC)
    with tile.TileContext(nc) as tc:
        with (
            tc.tile_pool(name="sbuf", bufs=1) as pool,
            tc.tile_pool(name="psum", bufs=1, space="PSUM") as psum,
        ):
            wgt = pool.tile([P, KD, E], f16)
            nc.sync.dma_start(wgt[:], wg.ap().rearrange("(ko p) e -> p ko e", p=P))
            bgt = pool.tile([E, 1], f32)
            nc.sync.dma_start(bgt[:], bg.ap()[:])
            xks = [pool.tile([P, TPC], f16, name=f"x{k}") for k in range(KD)]
            for k in range(KD):
                nc.sync.dma_start(xks[k][:], xt.ap()[k * P:(k + 1) * P, :])
            pts = [psum.tile([E, 512], f32, name=f"pg{si}") for si in range(len(subs))]
            for k in range(KD):
                for si, (s0, sz) in enumerate(subs):
                    nc.tensor.matmul(
                        pts[si][:, :sz], wgt[:, k], xks[k][:, s0:s0 + sz],
                        start=(k == 0), stop=(k == KD - 1),
                    )
            for si, (s0, sz) in enumerate(subs):
                ls = pool.tile([E, 512], f32, name=f"ls{si}")
                nc.vector.tensor_scalar_add(ls[:, :sz], pts[si][:, :sz], bgt[:, 0:1])
                nc.sync.dma_start(lo.ap()[:, s0:s0 + sz], ls[:, :sz])
    nc.compile()
    _cache["gate"] = nc
    return nc


def _build_ffn(segs):
    """segs: tuple of segment sizes; each segment is served by its own
    expert weight set (inputs w1_<s>, b1_<s>, w2_<s>, b2_<s>)."""
    key = ("ffn", segs)
    if key in _cache:
        return _cache[key]
    f32 = mybir.dt.float32
    f16 = mybir.dt.float16
    C = sum(segs)
    nseg = len(segs)
    seg0 = [sum(segs[:s]) for s in range(nseg)]
    nc = bacc.Bacc("TRN2", target_bir_lowering=False, debug=False, num_devices=NC)
    xt = nc.dram_tensor("xt", (D, C), f16, kind="ExternalInput")
    ws = []
    for s in range(nseg):
        ws.append((
            nc.dram_tensor(f"w1_{s}", (KF, P, KD, P), f16, kind="ExternalInput"),
            nc.dram_tensor(f"b1_{s}", (P, KF), f32, kind="ExternalInput"),
            nc.dram_tensor(f"w2_{s}", (KD, P, KF, P), f16, kind="ExternalInput"),
            nc.dram_tensor(f"b2_{s}", (P, KD), f32, kind="ExternalInput"),
        ))
    yt = nc.dram_tensor("yt", (D, C), f32, kind="ExternalOutput")

    with tile.TileContext(nc) as tc:
        with (
            tc.tile_pool(name="xs", bufs=1) as xpool,
            tc.tile_pool(name="hs", bufs=1) as hpool,
            tc.tile_pool(name="w1p", bufs=6) as w1pool,
            tc.tile_pool(name="w2p", bufs=3) as w2pool,
            tc.tile_pool(name="yp", bufs=3) as ypool,
            tc.tile_pool(name="bp", bufs=1) as bpool,
        ):
            xks = [xpool.tile([P, C], f16, name=f"x{k}") for k in range(KD)]

            # DMA issue order matters: the PE needs w1[seg0,f=0..2] and the
            # first x k-slices as early as possible; everything else follows.
            w1t0 = w1pool.tile([P, KD, P], f16, name="w1t")
            nc.sync.dma_start(w1t0[:], ws[0][0].ap()[0])
            w1t1 = w1pool.tile([P, KD, P], f16, name="w1t")
            nc.sync.dma_start(w1t1[:], ws[0][0].ap()[1])
            nc.sync.dma_start(
                xks[0][:, 0:segs[0]], xt.ap()[0:P, 0:segs[0]]
            )
            w1t2 = w1pool.tile([P, KD, P], f16, name="w1t")
            nc.sync.dma_start(w1t2[:], ws[0][0].ap()[2])
            for k in range(1, KD):
                nc.sync.dma_start(
                    xks[k][:, 0:segs[0]], xt.ap()[k * P:(k + 1) * P, 0:segs[0]]
                )
            bts = []
            for s in range(nseg):
                b1t = bpool.tile([P, KF], f32, name=f"b1t{s}")
                nc.sync.dma_start(b1t[:], ws[s][1].ap()[:])
                b2t = bpool.tile([P, KD], f32, name=f"b2t{s}")
                nc.sync.dma_start(b2t[:], ws[s][3].ap()[:])
                bts.append((b1t, b2t))
            for s in range(1, nseg):
                for k in range(KD):
                    nc.sync.dma_start(
                        xks[k][:, seg0[s]:seg0[s] + segs[s]],
                        xt.ap()[k * P:(k + 1) * P, seg0[s]:seg0[s] + segs[s]],
                    )

            # per-segment H tiles (no cross-segment reuse hazards)
            hts = [
                [hpool.tile([P, segs[s]], f16, name=f"h{s}_{f}") for f in range(KF)]
                for s in range(nseg)
            ]
            preloaded = {(0, 0): w1t0, (0, 1): w1t1, (0, 2): w1t2}

            for s, sz_seg in enumerate(segs):
                w1_t, _, w2_t, _ = ws[s]
                b1t, b2t = bts[s]
                base = seg0[s]
                subs = _subchunks(sz_seg)

                # -- layer 1: H^T[f] = gelu(sum_k W1[k,f]^T X^T[k] + b1[f]) --
                with tc.tile_pool(name=f"ps1_{s}", bufs=2, space="PSUM") as psum1:
                    for f in range(KF):
                        if (s, f) in preloaded:
                            w1t = preloaded[(s, f)]
                        else:
                            w1t = w1pool.tile([P, KD, P], f16, name="w1t")
                            nc.sync.dma_start(w1t[:], w1_t.ap()[f])
                        pts = [
                            psum1.tile([P, 512], f32, name=f"p1_{si}")
                            for si in range(len(subs))
                        ]
                        for k in range(KD):
                            for si, (s0, sz) in enumerate(subs):
                                nc.tensor.matmul(
                                    pts[si][:, :sz], w1t[:, k],
                                    xks[k][:, base + s0:base + s0 + sz],
                                    start=(k == 0), stop=(k == KD - 1),
                                )
                        for si, (s0, sz) in enumerate(subs):
                            nc.scalar.activation(
                                hts[s][f][:, s0:s0 + sz], pts[si][:, :sz],
                                mybir.ActivationFunctionType.Gelu,
                                bias=b1t[:, f:f + 1], scale=1.0,
                            )

                # -- layer 2: Y^T[d] = sum_f W2[f,d]^T H^T[f] + b2[d] --
                with tc.tile_pool(name=f"ps2_{s}", bufs=2, space="PSUM") as psum2:
                    for d in range(KD):
                        w2t = w2pool.tile([P, KF, P], f16, name="w2t")
                        nc.sync.dma_start(w2t[:], w2_t.ap()[d])
                        pts = [
                            psum2.tile([P, 512], f32, name=f"p2_{si}")
                            for si in range(len(subs))
                        ]
                        for f in range(KF):
                            for si, (s0, sz) in enumerate(subs):
                                nc.tensor.matmul(
                                    pts[si][:, :sz], w2t[:, f],
                                    hts[s][f][:, s0:s0 + sz],
                                    start=(f == 0), stop=(f == KF - 1),
                                )
                        for si, (s0, sz) in enumerate(subs):
                            ys = ypool.tile([P, 512], f32, name="ysb")
                            nc.vector.tensor_scalar_add(
                                ys[:, :sz], pts[si][:, :sz], b2t[:, d:d + 1]
                            )
                            nc.sync.dma_start(
                                yt.ap()[d * P:(d + 1) * P, base + s0:base + s0 + sz],
                                ys[:, :sz],
                            )
    nc.compile()
    _cache[key] = nc
    return nc


def kernel(x, W1, b1, W2, b2, Wg, bg):
    x = np.asarray(x, dtype=np.float32)
    W1 = np.asarray(W1, dtype=np.float32)
    b1 = np.asarray(b1, dtype=np.float32)
    W2 = np.asarray(W2, dtype=np.float32)
    b2 = np.asarray(b2, dtype=np.float32)
    Wg = np.asarray(Wg, dtype=np.float32)
    bg = np.asarray(bg, dtype=np.float32)

    xf = x.reshape(T, D)
    XT16 = np.ascontiguousarray(xf.T.astype(np.float16))  # [D, T]

    # ---- pass 1: gate logits on device (fp16) ----
    nc1 = _build_gate()
    Wg16 = Wg.astype(np.float16)
    in_maps = [
        {
            "xt": np.ascontiguousarray(XT16[:, c * TPC:(c + 1) * TPC]),
            "wg": Wg16,
            "bg": bg.reshape(E, 1),
        }
        for c in range(NC)
    ]
    res1 = _run(nc1, in_maps)
    logits = np.concatenate(
        [res1.results[c]["lo"].T for c in range(NC)], axis=0
    )  # [T, E]
    idx = np.argmax(logits, axis=1)

    # exact host re-decision for tokens with a small top-2 margin
    srt = np.sort(logits, axis=1)
    margin = srt[:, -1] - srt[:, -2]
    close = np.nonzero(margin < GATE_MARGIN)[0]
    if len(close):
        gl = xf[close].astype(np.float64) @ Wg.astype(np.float64) + bg
        idx[close] = np.argmax(gl, axis=1)

    # ---- host routing: pair experts, 4 cores per pair, 2 segments/core ----
    toks = [np.nonzero(idx == e)[0] for e in range(E)]
    order = sorted(range(E), key=lambda e: -len(toks[e]))
    pairs = [(order[0], order[3]), (order[1], order[2])]
    GRP = NC // 2

    def _padto(n):
        return max(8, ((n + 7) // 8) * 8)

    S0 = _padto(max((len(toks[a]) + GRP - 1) // GRP for a, _ in pairs))
    S1 = _padto(max((len(toks[b]) + GRP - 1) // GRP for _, b in pairs))
    segs = (S0, S1)
    C = S0 + S1

    core_lists = []
    for g, (a, b) in enumerate(pairs):
        sa = np.array_split(toks[a], GRP)
        sb = np.array_split(toks[b], GRP)
        for i in range(GRP):
            core_lists.append((sa[i], sb[i]))

    W1d = W1.reshape(E, KD, P, KF, P).transpose(0, 3, 2, 1, 4)  # [E, KF, P, KD, P]
    W1d = np.ascontiguousarray(W1d).astype(np.float16)
    W2d = W2.reshape(E, KF, P, KD, P).transpose(0, 3, 2, 1, 4)  # [E, KD, P, KF, P]
    W2d = np.ascontiguousarray(W2d).astype(np.float16)
    b1d = np.ascontiguousarray(b1.reshape(E, KF, P).transpose(0, 2, 1))  # [E, P, KF]
    b2d = np.ascontiguousarray(b2.reshape(E, KD, P).transpose(0, 2, 1))  # [E, P, KD]

    in_maps2 = []
    for c in range(NC):
        ea, eb = pairs[c // GRP]
        la, lb = core_lists[c]
        tok = np.zeros(C, dtype=np.int64)
        tok[:len(la)] = la
        tok[S0:S0 + len(lb)] = lb
        in_maps2.append(
            {
                "xt": np.ascontiguousarray(XT16[:, tok]),
                "w1_0": W1d[ea], "b1_0": b1d[ea],
                "w2_0": W2d[ea], "b2_0": b2d[ea],
                "w1_1": W1d[eb], "b1_1": b1d[eb],
                "w2_1": W2d[eb], "b2_1": b2d[eb],
            }
        )

    nc2 = _build_ffn(segs)
    res2 = _run(nc2, in_maps2)

    out = np.empty((T, D), dtype=np.float32)
    for c in range(NC):
        la, lb = core_lists[c]
        yt_c = res2.results[c]["yt"]
        if len(la):
            out[la] = yt_c[:, :len(la)].T
        if len(lb):
            out[lb] = yt_c[:, S0:S0 + len(lb)].T
    return out.reshape(B, L, D)
